# revision 12
# baseline (speedup 1.0000x reference)
"""Trainium2 Bass kernel for nn_DifferentialGCNBlock (intra-spatial GCN + inter-frame GCN).

Sharding: 8 cores = 4 batches x 2 node-halves. Each core computes both GCN stages
fully locally for its (batch, 512-node half), using a 64-node halo on each side
(A_sp is a 3x3x3 stencil => bandwidth |i-j| <= 73 < 128). The inter-frame GCN's
path-graph adjacency is separable (dinv[f']*dinv[f]) and handled as a 3-point
frame stencil on DVE/ACT.

Per-core per-frame pipeline (all fp32, c-major activations matching DRAM layout):
  x^T[f] (C=256 part, M=640 free)  --DMA-->  SBUF
  P = x @ Wi            : lhsT = x^T slices (c,m), rhs = Wi (c,c')  -> P (m part, c')
  T^T = (A_sp @ P)^T    : lhsT = P slices (m,c'),  rhs = A band blocks (m,n)
  y = relu(T^T)         : ACT, PSUM->SBUF                       (c' part, n)
  Z^T = y^T @ Wo        : lhsT = Wo (c',c''), rhs = y            (c'' part, n)
  Zh = dinv[f]*Z^T      : ACT copy w/ scale, PSUM->SBUF
  out[f'] = relu(dinv[f']*(Zh[f'-1]+Zh[f']+Zh[f'+1]))  : DVE adds + ACT relu-scale
"""
import sys

for p in ("/opt/trn_rl_repo",):
    if p not in sys.path:
        sys.path.insert(0, p)

import numpy as np

H, W_, D = 16, 8, 8
N = H * W_ * D          # 1024
F = 25
C = 256
BS = 4
HALO = 64
NLOC = 512
M = HALO + NLOC + HALO  # 640
NCORES = 8

# banded structure of A_sp in local coords: m = n + 64 + delta, |delta| <= 73
# k-tile j (m in [128j, 128j+128)) touches n in [128j-137, 128j+137); widened
# to >=256 columns so float32r matmuls run at full rate (1 cycle/row).
_BANDS = [(0, 256), (0, 272), (112, 400), (240, 512), (256, 512)]
# order: j=2's band [112,400) overlaps every other band, so putting it first
# (start=True) guarantees the whole-bank has_written clear happens first; the
# remaining matmuls accumulate per-element and are order-independent.
_BAND_ORDER = [2, 0, 1, 3, 4]


def _build_program():
    import concourse.bass as bass
    import concourse.tile as tile
    from concourse import bacc, mybir

    f32 = mybir.dt.float32
    f32r = mybir.dt.float32r  # single-pass reduced-precision fp32 matmul
    AF = mybir.ActivationFunctionType
    ALU = mybir.AluOpType

    # frame-graph normalization (path graph + self loops): deg = 2 at ends, 3 inside
    deg_fr = np.full(F, 3.0, np.float32)
    deg_fr[0] = deg_fr[F - 1] = 2.0
    dinv = (1.0 / np.sqrt(deg_fr)).astype(np.float32)

    # Bacc (not plain Bass): its compile pipeline splits multi-waits into
    # event semaphores (TRN2 allows at most 1 sync wait per instruction).
    nc = bacc.Bacc(None, target_bir_lowering=False, debug=False)
    x_in = nc.declare_dram_parameter("x", [F, C, M], f32r, isOutput=False)
    a_in = nc.declare_dram_parameter("A", [M, NLOC], f32r, isOutput=False)
    wi_in = nc.declare_dram_parameter("Wi", [C, C], f32r, isOutput=False)
    wo_in = nc.declare_dram_parameter("Wo", [C, C], f32r, isOutput=False)
    out_d = nc.declare_dram_parameter("out", [F, C, NLOC], f32, isOutput=True)

    with tile.TileContext(nc) as tc:
        with (
            tc.tile_pool(name="consts", bufs=1) as cpool,
            tc.tile_pool(name="xin", bufs=3) as xpool,
            tc.tile_pool(name="psb", bufs=2) as ppool,
            tc.tile_pool(name="ysb", bufs=2) as ypool,
            tc.tile_pool(name="zhsb", bufs=4) as zpool,
            tc.tile_pool(name="stmp", bufs=2) as spool,
            tc.tile_pool(name="osb", bufs=3) as opool,
            tc.tile_pool(name="pp", bufs=2, space="PSUM") as pp_ps,
            tc.tile_pool(name="pt", bufs=2, space="PSUM") as pt_ps,
            tc.tile_pool(name="pz", bufs=1, space="PSUM") as pz_ps,
        ):
            # ---- constants into SBUF ----
            a_sb = []
            for j in range(5):
                t = cpool.tile([128, NLOC], f32r, tag=f"A{j}")
                nc.sync.dma_start(out=t[:], in_=a_in[128 * j : 128 * (j + 1), :])
                a_sb.append(t)
            wi_sb = []
            wo_sb = []
            for kc in range(2):
                t = cpool.tile([128, C], f32r, tag=f"Wi{kc}")
                nc.sync.dma_start(out=t[:], in_=wi_in[128 * kc : 128 * (kc + 1), :])
                wi_sb.append(t)
                t = cpool.tile([128, C], f32r, tag=f"Wo{kc}")
                nc.sync.dma_start(out=t[:], in_=wo_in[128 * kc : 128 * (kc + 1), :])
                wo_sb.append(t)

            zh = {}  # (f, co) -> tile

            def emit_out(fp):
                for co in range(2):
                    terms = [zh[(fp, co)]]
                    if fp > 0:
                        terms.append(zh[(fp - 1, co)])
                    if fp < F - 1:
                        terms.append(zh[(fp + 1, co)])
                    if len(terms) == 3:
                        t1 = spool.tile([128, NLOC], f32, tag=f"st{co}")
                        nc.vector.tensor_add(t1[:], terms[1][:], terms[2][:])
                        t2 = spool.tile([128, NLOC], f32, tag=f"su{co}")
                        nc.vector.tensor_add(t2[:], t1[:], terms[0][:])
                    else:
                        t2 = spool.tile([128, NLOC], f32, tag=f"su{co}")
                        nc.vector.tensor_add(t2[:], terms[0][:], terms[1][:])
                    o = opool.tile([128, NLOC], f32, tag=f"o{co}")
                    # relu(s * dinv) on the otherwise-idle GpSimd engine
                    nc.gpsimd.tensor_scalar(
                        o[:], t2[:], float(dinv[fp]), 0.0, ALU.mult, ALU.max
                    )
                    nc.sync.dma_start(
                        out=out_d[fp, 128 * co : 128 * (co + 1), :], in_=o[:]
                    )

            for f in range(F):
                # ---- load x^T[f] ----
                xt = []
                for ct in range(2):
                    t = xpool.tile([128, M], f32r, tag=f"x{ct}")
                    nc.sync.dma_start(
                        out=t[:], in_=x_in[f, 128 * ct : 128 * (ct + 1), :]
                    )
                    xt.append(t)
                # ---- stage 1a: P (m part, c') ----
                p_sb = []
                for mi in range(5):
                    ps = pp_ps.tile([128, C], f32, tag="pp")
                    for kc in range(2):
                        nc.tensor.matmul(
                            ps[:],
                            xt[kc][:, 128 * mi : 128 * (mi + 1)],
                            wi_sb[kc][:],
                            start=(kc == 0),
                            stop=(kc == 1),
                        )
                    sb = ppool.tile([128, C], f32r, tag=f"p{mi}")
                    # split PSUM evacuation between DVE and ACT
                    if mi % 2 == 0:
                        nc.vector.tensor_copy(sb[:], ps[:])
                    else:
                        nc.scalar.copy(sb[:], ps[:])
                    p_sb.append(sb)
                # ---- stage 1c: T^T (c' part, n), banded accumulation ----
                y_sb = []
                for cp in range(2):
                    ts = pt_ps.tile([128, NLOC], f32, tag=f"t{cp}")
                    for oi, j in enumerate(_BAND_ORDER):
                        # group opener spans the full bank (A is zero outside
                        # its band) so later banded matmuls purely accumulate
                        n0, n1 = (0, NLOC) if oi == 0 else _BANDS[j]
                        nc.tensor.matmul(
                            ts[:, n0:n1],
                            p_sb[j][:, 128 * cp : 128 * (cp + 1)],
                            a_sb[j][:, n0:n1],
                            start=(oi == 0),
                            stop=(oi == 4),
                            skip_group_check=True,
                        )
                    yb = ypool.tile([128, NLOC], f32r, tag=f"y{cp}")
                    nc.scalar.activation(yb[:], ts[:], AF.Relu)
                    y_sb.append(yb)
                # ---- stage 2a: Z^T (c'' part, n) ----
                for co in range(2):
                    zs = pz_ps.tile([128, NLOC], f32, tag=f"z{co}")
                    for kc in range(2):
                        nc.tensor.matmul(
                            zs[:],
                            wo_sb[kc][:, 128 * co : 128 * (co + 1)],
                            y_sb[kc][:],
                            start=(kc == 0),
                            stop=(kc == 1),
                        )
                    zt = zpool.tile([128, NLOC], f32, tag=f"zh{co}")
                    nc.scalar.activation(zt[:], zs[:], AF.Copy, scale=float(dinv[f]))
                    zh[(f, co)] = zt
                # ---- stage 2b: emit frame f-1 (needs Zh[f]) ----
                if f >= 1:
                    emit_out(f - 1)
            emit_out(F - 1)

    # run the bacc compile pipeline (multi-wait splitting via event semaphores,
    # register allocation) — the axon SPMD exec path doesn't finalize for us
    nc.finalize()
    return nc


_CACHED = {}


def _get_program():
    if "nc" not in _CACHED:
        _CACHED["nc"] = _build_program()
    return _CACHED["nc"]


def kernel(d_seq, W_intra, W_inter, adj_space, adj_frame):
    from concourse.bass_utils import run_bass_kernel_spmd

    d_seq = np.asarray(d_seq, dtype=np.float32)
    W_intra = np.asarray(W_intra, dtype=np.float32)
    W_inter = np.asarray(W_inter, dtype=np.float32)
    adj_space = np.asarray(adj_space, dtype=np.float32)

    # host-side normalization of the spatial adjacency (tiny, deterministic)
    deg = adj_space.sum(-1)
    dinv_sp = 1.0 / np.sqrt(deg)
    A_sp = (adj_space * dinv_sp[:, None] * dinv_sp[None, :]).astype(np.float32)

    nc = _get_program()

    in_maps = []
    for core in range(NCORES):
        b, half = divmod(core, 2)
        own_lo = half * NLOC
        g_lo, g_hi = own_lo - HALO, own_lo + NLOC + HALO
        v_lo, v_hi = max(0, g_lo), min(N, g_hi)
        x_sl = np.zeros((F, C, M), dtype=np.float32)
        x_sl[:, :, v_lo - g_lo : v_hi - g_lo] = d_seq[b].reshape(F, C, N)[:, :, v_lo:v_hi]
        A_sl = np.zeros((M, NLOC), dtype=np.float32)
        A_sl[v_lo - g_lo : v_hi - g_lo, :] = A_sp[v_lo:v_hi, own_lo : own_lo + NLOC]
        in_maps.append(
            {
                "x": np.ascontiguousarray(x_sl),
                "A": np.ascontiguousarray(A_sl),
                "Wi": np.ascontiguousarray(W_intra),
                "Wo": np.ascontiguousarray(W_inter),
            }
        )

    res = run_bass_kernel_spmd(nc, in_maps, list(range(NCORES)))

    out = np.zeros((BS, F, C, N), dtype=np.float32)
    for core in range(NCORES):
        b, half = divmod(core, 2)
        own_lo = half * NLOC
        out[b, :, :, own_lo : own_lo + NLOC] = res.results[core]["out"]
    return out.reshape(d_seq.shape)


# revision 13
# speedup vs baseline: 2.7040x; 2.7040x over previous
"""Trainium2 Bass kernel for nn_DifferentialGCNBlock (intra-spatial GCN + inter-frame GCN).

Sharding: 8 cores = 4 batches x 2 node-halves. Each core computes both GCN stages
fully locally for its (batch, 512-node half), using a 64-node halo on each side
(A_sp is a 3x3x3 stencil => bandwidth |i-j| <= 73 < 128). The inter-frame GCN's
path-graph adjacency is separable (dinv[f']*dinv[f]) and handled as a 3-point
frame stencil on DVE/ACT.

Per-core per-frame pipeline (all fp32, c-major activations matching DRAM layout):
  x^T[f] (C=256 part, M=640 free)  --DMA-->  SBUF
  P = x @ Wi            : lhsT = x^T slices (c,m), rhs = Wi (c,c')  -> P (m part, c')
  T^T = (A_sp @ P)^T    : lhsT = P slices (m,c'),  rhs = A band blocks (m,n)
  y = relu(T^T)         : ACT, PSUM->SBUF                       (c' part, n)
  Z^T = y^T @ Wo        : lhsT = Wo (c',c''), rhs = y            (c'' part, n)
  Zh = dinv[f]*Z^T      : ACT copy w/ scale, PSUM->SBUF
  out[f'] = relu(dinv[f']*(Zh[f'-1]+Zh[f']+Zh[f'+1]))  : DVE adds + ACT relu-scale
"""
import sys

for p in ("/opt/trn_rl_repo",):
    if p not in sys.path:
        sys.path.insert(0, p)

import numpy as np

H, W_, D = 16, 8, 8
N = H * W_ * D          # 1024
F = 25
C = 256
BS = 4
HALO = 64
NLOC = 512
M = HALO + NLOC + HALO  # 640
NCORES = 8

# banded structure of A_sp in local coords: m = n + 64 + delta, |delta| <= 73
# k-tile j (m in [128j, 128j+128)) touches n in [128j-137, 128j+137); widened
# to >=256 columns so float32r matmuls run at full rate (1 cycle/row).
_BANDS = [(0, 256), (0, 272), (112, 400), (240, 512), (256, 512)]
# order: j=2's band [112,400) overlaps every other band, so putting it first
# (start=True) guarantees the whole-bank has_written clear happens first; the
# remaining matmuls accumulate per-element and are order-independent.
_BAND_ORDER = [2, 0, 1, 3, 4]


def _build_program():
    import concourse.bass as bass
    import concourse.tile as tile
    from concourse import bacc, mybir

    f32 = mybir.dt.float32
    f32r = mybir.dt.float32r  # single-pass reduced-precision fp32 matmul
    bf16 = mybir.dt.bfloat16
    AF = mybir.ActivationFunctionType
    ALU = mybir.AluOpType

    # frame-graph normalization (path graph + self loops): deg = 2 at ends, 3 inside
    deg_fr = np.full(F, 3.0, np.float32)
    deg_fr[0] = deg_fr[F - 1] = 2.0
    dinv = (1.0 / np.sqrt(deg_fr)).astype(np.float32)

    # Bacc (not plain Bass): its compile pipeline splits multi-waits into
    # event semaphores (TRN2 allows at most 1 sync wait per instruction).
    nc = bacc.Bacc(None, target_bir_lowering=False, debug=False)
    x_in = nc.declare_dram_parameter("x", [F, C, M], f32r, isOutput=False)
    a_in = nc.declare_dram_parameter("A", [M, NLOC], f32r, isOutput=False)
    wi_in = nc.declare_dram_parameter("Wi", [C, C], f32r, isOutput=False)
    wo_in = nc.declare_dram_parameter("Wo", [C, C], f32r, isOutput=False)
    out_d = nc.declare_dram_parameter("out", [F, C, NLOC], f32, isOutput=True)

    with tile.TileContext(nc) as tc:
        with (
            tc.tile_pool(name="consts", bufs=1) as cpool,
            tc.tile_pool(name="xin", bufs=4) as xpool,
            tc.tile_pool(name="psb", bufs=2) as ppool,
            tc.tile_pool(name="ysb", bufs=2) as ypool,
            tc.tile_pool(name="zhsb", bufs=4) as zpool,
            tc.tile_pool(name="stmp", bufs=2) as spool,
            tc.tile_pool(name="osb", bufs=3) as opool,
            tc.tile_pool(name="pp", bufs=2, space="PSUM") as pp_ps,
            tc.tile_pool(name="pt", bufs=2, space="PSUM") as pt_ps,
            tc.tile_pool(name="pz", bufs=1, space="PSUM") as pz_ps,
        ):
            # ---- constants into SBUF ----
            a_sb = []
            for j in range(5):
                t = cpool.tile([128, NLOC], f32r, tag=f"A{j}")
                nc.sync.dma_start(out=t[:], in_=a_in[128 * j : 128 * (j + 1), :])
                a_sb.append(t)
            wi_sb = []
            wo_sb = []
            for kc in range(2):
                t = cpool.tile([128, C], f32r, tag=f"Wi{kc}")
                nc.sync.dma_start(out=t[:], in_=wi_in[128 * kc : 128 * (kc + 1), :])
                wi_sb.append(t)
                t = cpool.tile([128, C], f32r, tag=f"Wo{kc}")
                nc.sync.dma_start(out=t[:], in_=wo_in[128 * kc : 128 * (kc + 1), :])
                wo_sb.append(t)

            zh = {}  # f -> (128, 2, NLOC) bf16 tile

            def emit_out(fp):
                # 3-point frame stencil in bf16 (DVE 2x mode), wide ops over both
                # c-tile halves at once; relu * dinv on ACT writes fp32 out
                terms = [zh[fp]]
                if fp > 0:
                    terms.append(zh[fp - 1])
                if fp < F - 1:
                    terms.append(zh[fp + 1])
                if len(terms) == 3:
                    t1 = spool.tile([128, 2, NLOC], bf16, tag="st")
                    nc.vector.tensor_add(t1[:], terms[1][:], terms[2][:])
                    t2 = spool.tile([128, 2, NLOC], bf16, tag="su")
                    nc.vector.tensor_add(t2[:], t1[:], terms[0][:])
                else:
                    t2 = spool.tile([128, 2, NLOC], bf16, tag="su")
                    nc.vector.tensor_add(t2[:], terms[0][:], terms[1][:])
                o = opool.tile([128, 2, NLOC], f32, tag="o")
                nc.scalar.activation(o[:], t2[:], AF.Relu, scale=float(dinv[fp]))
                for co in range(2):
                    nc.sync.dma_start(
                        out=out_d[fp, 128 * co : 128 * (co + 1), :], in_=o[:, co, :]
                    )

            for f in range(F):
                # ---- load x^T[f] ----
                xt = []
                for ct in range(2):
                    t = xpool.tile([128, M], f32r, tag=f"x{ct}")
                    nc.sync.dma_start(
                        out=t[:], in_=x_in[f, 128 * ct : 128 * (ct + 1), :]
                    )
                    xt.append(t)
                # ---- stage 1a: P (m part, c') ----
                p_sb = []
                for mi in range(5):
                    ps = pp_ps.tile([128, C], f32, tag="pp")
                    for kc in range(2):
                        nc.tensor.matmul(
                            ps[:],
                            xt[kc][:, 128 * mi : 128 * (mi + 1)],
                            wi_sb[kc][:],
                            start=(kc == 0),
                            stop=(kc == 1),
                        )
                    sb = ppool.tile([128, C], f32r, tag=f"p{mi}")
                    nc.vector.tensor_copy(sb[:], ps[:])
                    p_sb.append(sb)
                # ---- stage 1c: T^T (c' part, n), banded accumulation ----
                ts = pt_ps.tile([128, 2, NLOC], f32, tag="t")
                for cp in range(2):
                    for oi, j in enumerate(_BAND_ORDER):
                        # group opener spans the full bank (A is zero outside
                        # its band) so later banded matmuls purely accumulate
                        n0, n1 = (0, NLOC) if oi == 0 else _BANDS[j]
                        nc.tensor.matmul(
                            ts[:, cp, n0:n1],
                            p_sb[j][:, 128 * cp : 128 * (cp + 1)],
                            a_sb[j][:, n0:n1],
                            start=(oi == 0),
                            stop=(oi == 4),
                            skip_group_check=True,
                        )
                yb = ypool.tile([128, 2, NLOC], f32r, tag="y")
                nc.scalar.activation(yb[:], ts[:], AF.Relu)
                # ---- stage 2a: Z^T (c'' part, n) ----
                zs = pz_ps.tile([128, 2, NLOC], f32, tag="z")
                for co in range(2):
                    for kc in range(2):
                        nc.tensor.matmul(
                            zs[:, co, :],
                            wo_sb[kc][:, 128 * co : 128 * (co + 1)],
                            yb[:, kc, :],
                            start=(kc == 0),
                            stop=(kc == 1),
                        )
                zt = zpool.tile([128, 2, NLOC], bf16, tag="zh")
                nc.scalar.activation(zt[:], zs[:], AF.Copy, scale=float(dinv[f]))
                zh[f] = zt
                # ---- stage 2b: emit frame f-1 (needs Zh[f]) ----
                if f >= 1:
                    emit_out(f - 1)
            emit_out(F - 1)

    # run the bacc compile pipeline (multi-wait splitting via event semaphores,
    # register allocation) — the axon SPMD exec path doesn't finalize for us
    nc.finalize()
    return nc


_CACHED = {}


def _get_program():
    if "nc" not in _CACHED:
        _CACHED["nc"] = _build_program()
    return _CACHED["nc"]


def kernel(d_seq, W_intra, W_inter, adj_space, adj_frame):
    from concourse.bass_utils import run_bass_kernel_spmd

    d_seq = np.asarray(d_seq, dtype=np.float32)
    W_intra = np.asarray(W_intra, dtype=np.float32)
    W_inter = np.asarray(W_inter, dtype=np.float32)
    adj_space = np.asarray(adj_space, dtype=np.float32)

    # host-side normalization of the spatial adjacency (tiny, deterministic)
    deg = adj_space.sum(-1)
    dinv_sp = 1.0 / np.sqrt(deg)
    A_sp = (adj_space * dinv_sp[:, None] * dinv_sp[None, :]).astype(np.float32)

    nc = _get_program()

    in_maps = []
    for core in range(NCORES):
        b, half = divmod(core, 2)
        own_lo = half * NLOC
        g_lo, g_hi = own_lo - HALO, own_lo + NLOC + HALO
        v_lo, v_hi = max(0, g_lo), min(N, g_hi)
        x_sl = np.zeros((F, C, M), dtype=np.float32)
        x_sl[:, :, v_lo - g_lo : v_hi - g_lo] = d_seq[b].reshape(F, C, N)[:, :, v_lo:v_hi]
        A_sl = np.zeros((M, NLOC), dtype=np.float32)
        A_sl[v_lo - g_lo : v_hi - g_lo, :] = A_sp[v_lo:v_hi, own_lo : own_lo + NLOC]
        in_maps.append(
            {
                "x": np.ascontiguousarray(x_sl),
                "A": np.ascontiguousarray(A_sl),
                "Wi": np.ascontiguousarray(W_intra),
                "Wo": np.ascontiguousarray(W_inter),
            }
        )

    res = run_bass_kernel_spmd(nc, in_maps, list(range(NCORES)))

    out = np.zeros((BS, F, C, N), dtype=np.float32)
    for core in range(NCORES):
        b, half = divmod(core, 2)
        own_lo = half * NLOC
        out[b, :, :, own_lo : own_lo + NLOC] = res.results[core]["out"]
    return out.reshape(d_seq.shape)
